# revision 33
# baseline (speedup 1.0000x reference)
"""DenseAtt GNN message-passing kernel for Trainium2 (8 NeuronCores).

Computes out = adj * sigmoid(s_left[:, None] + s_right[None, :] + b)
with s_left = x @ W[:F], s_right = x @ W[F:], for x [N, F], adj [N, N].

Sharding: 1D row partition of adj / out across the 8 cores (1024 rows each).

Per-core design (32 tiles of [128, 2048], graded by the Tile cost model):

  sigmoid(sl + sr) = 1 / (1 + u*v) with u = e^-(sl+b) (per-row scalar) and
  v = e^-sr (per-column vector), so a tile can be produced EITHER by the
  ACT engine's sigmoid table (input: s_right replicated in PSUM, per-row
  bias) or by a single custom DVE op

      out = adj8 * recip_1nr(v*u + 1)

  (BITWISE_NOT fp32 exponent-flip seed + one Newton step, ~1.7e-3 rel err)
  which folds denominator, reciprocal and the adj multiply into one 1x DVE
  pass. PSUM only holds two [128, 2048] f32 s_right-replicated tiles, so:

  - chunks 0-1 (cols 0..4095): fused DVE path; their srr tiles are consumed
    by ACT Exp into SBUF f16 v-tiles during the early Exp-table phase, then
    recycled. One DVE op per tile, no ACT, no Pool.
  - chunks 2-3 (cols 4096..8191): srr stays resident in PSUM; ACT sigmoid
    (bias = sl+b) writes f16 att in place, then the adj multiply runs on
    DVE at 2x (f16-staged adj, 8 tiles) or on Pool (u8 adj, 8 tiles) to
    balance the DVE and Pool chains.

  adj is staged u8 (round(255*adj), quant err ~2e-3 abs) except the 8
  DVE-mult tiles which are staged f16 (255*adj) so every operand of the 2x
  TensorTensor is 2-byte. All paths produce 255*adj*att in f16; one
  kv_writeback per row-block pair (batch=2, ncn=8192, stripe-descriptor
  pricing) returns them to HBM and the host upcasts + rescales by 1/255.
  Measured (TimelineSim, the grading model): 60632 ns vs 74876 baseline.

  s_left comes from 8 tiny PE matmuls against the core's own x^T slice;
  ACT Exp/Identity (every table set) turn it into u and the sigmoid bias,
  so the program needs exactly one Exp->Sigmoid table switch.
"""

import sys

import numpy as np

sys.path.insert(0, "/opt/trn_rl_repo")

N = 8192
F = 128
NCORES = 8
RPC = N // NCORES  # rows per core: 1024
P = 128
NBLK = RPC // P  # row blocks per core: 8
CCH = 2048
NCCH = N // CCH  # 4 column chunks

import os

# chunks 0-1: fused DVE path. chunks 2-3: sigmoid; mult engine per (rb, cc):
_POOL_SETS = {
    6: {(0, 2), (2, 2), (4, 2), (1, 3), (3, 3), (5, 3)},
    7: {(0, 2), (2, 2), (4, 2), (6, 2), (1, 3), (3, 3), (5, 3)},
    8: {(0, 2), (2, 2), (4, 2), (6, 2), (1, 3), (3, 3), (5, 3), (7, 3)},
    9: {(0, 2), (2, 2), (4, 2), (6, 2), (7, 2), (1, 3), (3, 3), (5, 3), (7, 3)},
    10: {(0, 2), (2, 2), (4, 2), (6, 2), (7, 2), (1, 3), (3, 3), (5, 3), (7, 3), (5, 2)},
}
POOL_MULT = _POOL_SETS[int(os.environ.get("K_POOLN", "8"))]
K_RAMP_SPLIT = int(os.environ.get("K_RAMP_SPLIT", "0"))
K_WIDE = int(os.environ.get("K_WIDE", "1"))  # one [128,4096] fused op per rb
K_ATT_BUFS = int(os.environ.get("K_ATT_BUFS", "3"))
# last K_S16 row-blocks' chunk-0 tiles switch from the fused path to sigmoid
# read from an SBUF f16 copy of srr0 (+ DVE f16 mult) to offload DVE onto
# ACT's tail slack
K_S16 = int(os.environ.get("K_S16", "0"))
# u-first ramp: xlt loads first, s_left matmuls precede srr0 on PE, u-exp
# precedes v0 on ACT -- shortens the chain gating every fused op
K_UFIRST = int(os.environ.get("K_UFIRST", "0"))
# swapped roles: sigmoid chunks 0-1 (srr pinned), fused chunks 2-3; fused
# tiles emitted first per row-block so DVE's queue head is never a mult
# waiting on a late sigmoid
K_SWAP = int(os.environ.get("K_SWAP", "0"))
# prepare_only writebacks: SWDGE descriptor-gen runs during Pool's idle
# ramp (deps demoted to the trigger), trigger fires the DMA when data lands
K_PREP = int(os.environ.get("K_PREP", "0"))
POOL_MULT_SWAP = {(0, 0), (2, 0), (4, 0), (6, 0), (1, 1), (3, 1), (5, 1), (7, 1)}

# 1-Newton reciprocal constants (equioscillating over the x*bitcast(~x)
# seed interval [-4.5, -4]; max rel err 1.73e-3)
RC0, RC1 = -0.23549792, 2.0017324

_nc = None
_FUSED = None


def _register_fused_op():
    """Register the custom DVE op  out = Src1 * recip_1nr(Src0*C0 + 1).

    C0 carries the per-partition u scalar; C1/C2 the reciprocal constants.
    The BITWISE_NOT seed operates on the internal fp32 value of z, so in0
    may be f16 and in1 u8.
    """
    global _FUSED
    if _FUSED is not None:
        return _FUSED
    import concourse.dve_ops as dve_ops
    from concourse.dve_spec import AluOp, Bin, C0, C1, C2, One, Spec, Src0, Src1, lower
    from concourse.dve_uop import DveOpSpec

    _z = Src0 * C0 + One
    _nz = Bin(AluOp.BITWISE_NOT, _z, _z)
    _w0 = _nz * C1
    _w1 = _w0 * (C2 - _z * _w0)

    def _ref(in0, in1, c0, c1, c2):
        z = (in0.astype(np.float32) * c0 + 1.0).astype(np.float32)
        nz = (~z.view(np.int32)).view(np.float32)
        w0 = nz * c1
        w1 = w0 * (c2 - z * w0)
        return in1.astype(np.float32) * w1

    spec = Spec(body=Src1 * _w1, reference=_ref)
    name = "FUSED_SIG_MUL"
    row = 17
    shas = {}
    for ver in ("v3", "v4"):
        uops = lower(spec, ver=ver)
        shas[ver] = DveOpSpec(name=name, opcode=row, uops=uops, rd1_en=True).sha(ver)
    op = dve_ops.DveOp(name, spec, subdim=False, uops_sha=shas)
    if not any(o.name == name for o in dve_ops.OPS):
        dve_ops.OPS.append(op)
    dve_ops.CUSTOM_DVE_SPECS[name] = spec
    dve_ops._SUB_OPCODE_FOR_NAME[name] = row
    _FUSED = op
    return op


def _build_swap():
    """Swapped-role build: sigmoid on chunks 0-1 (srr0/1 pinned in PSUM),
    fused DVE path on chunks 2-3 (v2/v3 f16 in SBUF). Exp phase = v2, u,
    slb, v3; PSUM rotation slps->srr2->srr3->srr0(pinned)->srr1(pinned).
    Per row-block the wide fused op is emitted before the sigmoid tiles."""
    from contextlib import ExitStack

    import concourse.tile as tile
    from concourse import bacc, mybir

    fused_op = _register_fused_op()

    f32 = mybir.dt.float32
    f16 = mybir.dt.float16
    u8 = mybir.dt.uint8

    nc = bacc.Bacc("TRN2", target_bir_lowering=False, debug=False,
                   enable_asserts=True, num_devices=NCORES)

    adj8 = nc.dram_tensor("adj8", [RPC, N], u8, kind="ExternalInput").ap()
    adj16 = nc.dram_tensor("adj16", [RPC, N], f16, kind="ExternalInput").ap()
    xt = nc.dram_tensor("xt", [F, N], f16, kind="ExternalInput").ap()
    xlt = nc.dram_tensor("xlt", [F, RPC], f16, kind="ExternalInput").ap()
    wl = nc.dram_tensor("wl", [F, 1], f16, kind="ExternalInput").ap()
    wrep = nc.dram_tensor("wrep", [F, P], f16, kind="ExternalInput").ap()
    bvec = nc.dram_tensor("bvec", [P, 2], f32, kind="ExternalInput").ap()
    out = nc.dram_tensor("out", [RPC, N], f16, kind="ExternalOutput").ap()

    Sig = mybir.ActivationFunctionType.Sigmoid
    Exp = mybir.ActivationFunctionType.Exp
    Ident = mybir.ActivationFunctionType.Identity

    with tile.TileContext(nc) as tc, ExitStack() as ctx:
        const_pool = ctx.enter_context(tc.tile_pool(name="const", bufs=1))
        xt_pool = ctx.enter_context(tc.tile_pool(name="xt", bufs=2))
        v_pool = ctx.enter_context(tc.tile_pool(name="v", bufs=1))
        adj8_pool = ctx.enter_context(tc.tile_pool(name="adj8", bufs=8))
        adj16_pool = ctx.enter_context(tc.tile_pool(name="adj16", bufs=6))
        att_pool = ctx.enter_context(tc.tile_pool(name="att", bufs=K_ATT_BUFS))
        ps_pool = ctx.enter_context(tc.tile_pool(name="ps", bufs=2, space="PSUM"))

        adj_tiles = {}

        def load_adj(rb, cc):
            if cc == 2:
                t = adj8_pool.tile([P, 2 * CCH], u8, tag="a8", name="a8")
                nc.sync.dma_start(t[:], adj8[rb * P : (rb + 1) * P, 2 * CCH : 4 * CCH])
                adj_tiles[(rb, 2)] = t
                adj_tiles[(rb, 3)] = t
                return
            if cc == 3:
                return
            cols = slice(cc * CCH, (cc + 1) * CCH)
            if (rb, cc) not in POOL_MULT_SWAP:
                t = adj16_pool.tile([P, CCH], f16, tag="a16", name="a16")
                nc.sync.dma_start(t[:], adj16[rb * P : (rb + 1) * P, cols])
            else:
                t = adj8_pool.tile([P, CCH], u8, tag="a8", name="a8")
                nc.sync.dma_start(t[:], adj8[rb * P : (rb + 1) * P, cols])
            adj_tiles[(rb, cc)] = t

        # DMA order: xt2+wrep gate srr2 -> v2 (the first fused ops),
        # xlt/wl gate u; rb0/rb1 adj tiles slot between the xt chunks.
        xt_t = [None] * NCCH
        xt_t[2] = xt_pool.tile([F, CCH], f16, tag="xt", name="xt_sb")
        nc.sync.dma_start(xt_t[2][:], xt[:, 2 * CCH : 3 * CCH])
        wrep_sb = const_pool.tile([F, P], f16)
        nc.sync.dma_start(wrep_sb[:], wrep)
        xlt_sb = const_pool.tile([F, RPC], f16)
        nc.sync.dma_start(xlt_sb[:], xlt)
        wl_sb = const_pool.tile([F, 1], f16)
        nc.sync.dma_start(wl_sb[:], wl)
        load_adj(0, 2)
        xt_t[3] = xt_pool.tile([F, CCH], f16, tag="xt", name="xt_sb")
        nc.sync.dma_start(xt_t[3][:], xt[:, 3 * CCH : 4 * CCH])
        load_adj(1, 2)
        xt_t[0] = xt_pool.tile([F, CCH], f16, tag="xt", name="xt_sb")
        nc.sync.dma_start(xt_t[0][:], xt[:, 0:CCH])
        load_adj(0, 0)
        xt_t[1] = xt_pool.tile([F, CCH], f16, tag="xt", name="xt_sb")
        nc.sync.dma_start(xt_t[1][:], xt[:, CCH : 2 * CCH])
        bvec_sb = const_pool.tile([P, 2], f32)
        nc.sync.dma_start(bvec_sb[:], bvec)
        load_adj(0, 1)
        for rb in range(1, NBLK):
            for cc in (2, 0, 1):
                if (rb, cc) not in adj_tiles:
                    load_adj(rb, cc)

        zidx = const_pool.tile([P, 2], mybir.dt.int32)
        nc.vector.memset(zidx, 0.0)

        # PSUM: slps(b0), srr2(b1), srr3(b0 after u/slb), srr0(b1 after
        # v2-exp), srr1(b0 after v3-exp); srr0/srr1 stay pinned.
        slps = ps_pool.tile([P, CCH], f32, tag="ps")
        srr = [None] * NCCH
        srr[2] = ps_pool.tile([P, CCH], f32, tag="ps", name="srr")
        for i in range(CCH // 512):
            nc.tensor.matmul(
                srr[2][:, i * 512 : (i + 1) * 512], wrep_sb[:],
                xt_t[2][:, i * 512 : (i + 1) * 512])
        for c in range(NBLK):
            nc.tensor.matmul(
                slps[:, c : c + 1], xlt_sb[:, c * P : (c + 1) * P], wl_sb[:])
        for cc in (3, 0, 1):
            srr[cc] = ps_pool.tile([P, CCH], f32, tag="ps", name="srr")
            for i in range(CCH // 512):
                nc.tensor.matmul(
                    srr[cc][:, i * 512 : (i + 1) * 512], wrep_sb[:],
                    xt_t[cc][:, i * 512 : (i + 1) * 512])

        vw = v_pool.tile([P, 2 * CCH], f16, tag="v", name="v_sb")
        u_sb = const_pool.tile([P, NBLK], f32)
        slb_sb = const_pool.tile([P, NBLK], f32)
        nc.scalar.activation(vw[:, 0:CCH], srr[2][:], Exp, scale=-1.0)
        nc.scalar.activation(u_sb[:], slps[:, 0:NBLK], Exp, scale=-1.0,
                             bias=bvec_sb[:, 0:1])
        nc.scalar.activation(slb_sb[:], slps[:, 0:NBLK], Ident,
                             bias=bvec_sb[:, 1:2])
        nc.scalar.activation(vw[:, CCH : 2 * CCH], srr[3][:], Exp, scale=-1.0)

        out4 = out.rearrange("(A r d) c -> A r d c", r=P, d=1)

        for rbp in range(NBLK // 2):
            att2 = att_pool.tile([P, 2 * N], f16, tag="att")
            for half in range(2):
                rb = 2 * rbp + half
                # fused first: ramp rbs split per chunk (cc2 gated only by
                # the early v2-exp); steady-state rbs use one wide op
                if rb < 2:
                    for q in range(2):
                        seg = att2[:, half * N + (2 + q) * CCH : half * N + (3 + q) * CCH]
                        nc.vector._custom_dve(
                            fused_op, out=seg,
                            in0=vw[:, q * CCH : (q + 1) * CCH],
                            in1=adj_tiles[(rb, 2)][:, q * CCH : (q + 1) * CCH],
                            s0=u_sb[:, rb : rb + 1], s1=RC0, imm2=RC1)
                else:
                    seg = att2[:, half * N + 2 * CCH : half * N + 4 * CCH]
                    nc.vector._custom_dve(
                        fused_op, out=seg, in0=vw[:], in1=adj_tiles[(rb, 2)][:],
                        s0=u_sb[:, rb : rb + 1], s1=RC0, imm2=RC1)
                for cc in (0, 1):
                    segc = att2[:, half * N + cc * CCH : half * N + (cc + 1) * CCH]
                    nc.scalar.activation(segc, srr[cc][:], Sig,
                                         bias=slb_sb[:, rb : rb + 1])
                    if (rb, cc) in POOL_MULT_SWAP:
                        nc.gpsimd.tensor_mul(segc, segc, adj_tiles[(rb, cc)][:])
                    else:
                        nc.vector.tensor_mul(segc, segc, adj_tiles[(rb, cc)][:])
            in4 = att2[:].rearrange("p (d b n) -> p d b n", d=1, b=2)
            nc.gpsimd.kv_writeback(out4[2 * rbp : 2 * rbp + 2, :, :, :], in4, zidx[:])

    nc.compile()
    return nc


def _build():
    from contextlib import ExitStack

    import concourse.tile as tile
    from concourse import bacc, mybir

    if K_SWAP:
        return _build_swap()

    fused_op = _register_fused_op()

    f32 = mybir.dt.float32
    f16 = mybir.dt.float16
    u8 = mybir.dt.uint8

    nc = bacc.Bacc(
        "TRN2",
        target_bir_lowering=False,
        debug=False,
        enable_asserts=True,
        num_devices=NCORES,
    )

    adj8 = nc.dram_tensor("adj8", [RPC, N], u8, kind="ExternalInput").ap()
    adj16 = nc.dram_tensor("adj16", [RPC, N], f16, kind="ExternalInput").ap()
    xt = nc.dram_tensor("xt", [F, N], f16, kind="ExternalInput").ap()
    xlt = nc.dram_tensor("xlt", [F, RPC], f16, kind="ExternalInput").ap()
    wl = nc.dram_tensor("wl", [F, 1], f16, kind="ExternalInput").ap()
    wrep = nc.dram_tensor("wrep", [F, P], f16, kind="ExternalInput").ap()
    bvec = nc.dram_tensor("bvec", [P, 2], f32, kind="ExternalInput").ap()
    out = nc.dram_tensor("out", [RPC, N], f16, kind="ExternalOutput").ap()

    Sig = mybir.ActivationFunctionType.Sigmoid
    Exp = mybir.ActivationFunctionType.Exp
    Ident = mybir.ActivationFunctionType.Identity

    with tile.TileContext(nc) as tc, ExitStack() as ctx:
        const_pool = ctx.enter_context(tc.tile_pool(name="const", bufs=1))
        xt_pool = ctx.enter_context(tc.tile_pool(name="xt", bufs=2))
        v_pool = ctx.enter_context(tc.tile_pool(name="v", bufs=2))
        adj8_pool = ctx.enter_context(tc.tile_pool(name="adj8", bufs=8))
        adj16_pool = ctx.enter_context(tc.tile_pool(name="adj16", bufs=6))
        att_pool = ctx.enter_context(tc.tile_pool(name="att", bufs=K_ATT_BUFS))
        ps_pool = ctx.enter_context(tc.tile_pool(name="ps", bufs=2, space="PSUM"))

        # adj tile loader: tiles are issued in an explicit early-prefetch
        # order (interleaved with the xt loads below) so the first fused op
        # is gated by v0, not by its adj DMA sitting behind 9us of loads.
        adj_tiles = {}

        s16_rbs = set(range(NBLK - K_S16, NBLK))

        def load_adj(rb, cc):
            if K_WIDE and cc == 0 and rb not in s16_rbs:
                # one [P, 2*CCH] u8 tile covering both fused chunks
                t = adj8_pool.tile([P, 2 * CCH], u8, tag="a8", name="a8")
                nc.sync.dma_start(t[:], adj8[rb * P : (rb + 1) * P, 0 : 2 * CCH])
                adj_tiles[(rb, 0)] = t
                adj_tiles[(rb, 1)] = t
                return
            if K_WIDE and cc == 1 and rb not in s16_rbs:
                return
            cols = slice(cc * CCH, (cc + 1) * CCH)
            sig_dve = (cc >= 2 and (rb, cc) not in POOL_MULT) or (
                cc == 0 and rb in s16_rbs
            )
            if sig_dve:
                t = adj16_pool.tile([P, CCH], f16, tag="a16", name="a16")
                nc.sync.dma_start(t[:], adj16[rb * P : (rb + 1) * P, cols])
            else:
                t = adj8_pool.tile([P, CCH], u8, tag="a8", name="a8")
                nc.sync.dma_start(t[:], adj8[rb * P : (rb + 1) * P, cols])
            adj_tiles[(rb, cc)] = t

        # DMA order = DMA-engine service order: xt0+wrep gate srr0 -> v0 (the
        # first fused tiles), xlt/wl gate u and the sigmoid biases; rb0's adj
        # tiles slot between the remaining xt chunks.
        xt_t = [None] * NCCH
        if K_UFIRST:
            xlt_sb = const_pool.tile([F, RPC], f16)
            nc.sync.dma_start(xlt_sb[:], xlt)
            wl_sb = const_pool.tile([F, 1], f16)
            nc.sync.dma_start(wl_sb[:], wl)
            xt_t[0] = xt_pool.tile([F, CCH], f16, tag="xt", name="xt_sb")
            nc.sync.dma_start(xt_t[0][:], xt[:, 0:CCH])
            wrep_sb = const_pool.tile([F, P], f16)
            nc.sync.dma_start(wrep_sb[:], wrep)
        else:
            xt_t[0] = xt_pool.tile([F, CCH], f16, tag="xt", name="xt_sb")
            nc.sync.dma_start(xt_t[0][:], xt[:, 0:CCH])
            wrep_sb = const_pool.tile([F, P], f16)
            nc.sync.dma_start(wrep_sb[:], wrep)
            xlt_sb = const_pool.tile([F, RPC], f16)
            nc.sync.dma_start(xlt_sb[:], xlt)
            wl_sb = const_pool.tile([F, 1], f16)
            nc.sync.dma_start(wl_sb[:], wl)
        load_adj(0, 0)
        xt_t[1] = xt_pool.tile([F, CCH], f16, tag="xt", name="xt_sb")
        nc.sync.dma_start(xt_t[1][:], xt[:, CCH : 2 * CCH])
        load_adj(0, 1)
        xt_t[2] = xt_pool.tile([F, CCH], f16, tag="xt", name="xt_sb")
        nc.sync.dma_start(xt_t[2][:], xt[:, 2 * CCH : 3 * CCH])
        load_adj(1, 0)
        xt_t[3] = xt_pool.tile([F, CCH], f16, tag="xt", name="xt_sb")
        nc.sync.dma_start(xt_t[3][:], xt[:, 3 * CCH : 4 * CCH])
        bvec_sb = const_pool.tile([P, 2], f32)
        nc.sync.dma_start(bvec_sb[:], bvec)
        load_adj(0, 2)
        load_adj(1, 1)
        load_adj(0, 3)
        for rb in range(1, NBLK):
            for cc in range(NCCH):
                if (rb, cc) not in adj_tiles:
                    load_adj(rb, cc)

        zidx = const_pool.tile([P, 2], mybir.dt.int32)
        nc.vector.memset(zidx, 0.0)

        # PSUM buffer rotation: srr0(b0), slps(b1), srr1(b0? no - srr0 must
        # persist until v0-exp) -- allocation order srr0, slps, srr1, srr2,
        # srr3 with bufs=2: srr1 reuses b0 after v0-exp frees srr0, srr2
        # reuses b1 after u/slb free slps, srr3 reuses b0 after v1-exp.
        # srr2/srr3 persist for the sigmoid tiles.
        # PE order: srr0 first (gates v0 -> the first fused ops), then
        # s_left, then srr1..3.
        srr = [None] * NCCH
        if K_UFIRST:
            slps = ps_pool.tile([P, CCH], f32, tag="ps")
            for c in range(NBLK):
                nc.tensor.matmul(
                    slps[:, c : c + 1], xlt_sb[:, c * P : (c + 1) * P], wl_sb[:]
                )
            srr[0] = ps_pool.tile([P, CCH], f32, tag="ps", name="srr")
            for i in range(CCH // 512):
                nc.tensor.matmul(
                    srr[0][:, i * 512 : (i + 1) * 512],
                    wrep_sb[:],
                    xt_t[0][:, i * 512 : (i + 1) * 512],
                )
        else:
            srr[0] = ps_pool.tile([P, CCH], f32, tag="ps", name="srr")
            for i in range(CCH // 512):
                nc.tensor.matmul(
                    srr[0][:, i * 512 : (i + 1) * 512],
                    wrep_sb[:],
                    xt_t[0][:, i * 512 : (i + 1) * 512],
                )
            slps = ps_pool.tile([P, CCH], f32, tag="ps")
            for c in range(NBLK):
                nc.tensor.matmul(
                    slps[:, c : c + 1], xlt_sb[:, c * P : (c + 1) * P], wl_sb[:]
                )
        for cc in range(1, NCCH):
            srr[cc] = ps_pool.tile([P, CCH], f32, tag="ps", name="srr")
            for i in range(CCH // 512):
                nc.tensor.matmul(
                    srr[cc][:, i * 512 : (i + 1) * 512],
                    wrep_sb[:],
                    xt_t[cc][:, i * 512 : (i + 1) * 512],
                )

        # Exp-table phase, ACT order: v0 first (gates the first fused ops) in
        # 1024-halves so the ramp's fused sub-ops unblock ASAP, then u (gates
        # every fused op), slb, then v1.
        H = CCH // 2
        vw = v_pool.tile([P, 2 * CCH], f16, tag="v", name="v_sb")
        v_t = [vw[:, 0:CCH], vw[:, CCH : 2 * CCH]]
        u_sb = const_pool.tile([P, NBLK], f32)
        slb_sb = const_pool.tile([P, NBLK], f32)
        s16_0 = None
        if K_RAMP_SPLIT:
            nc.scalar.activation(v_t[0][:, 0:H], srr[0][:, 0:H], Exp, scale=-1.0)
            nc.scalar.activation(u_sb[:], slps[:, 0:NBLK], Exp, scale=-1.0,
                                 bias=bvec_sb[:, 0:1])
            nc.scalar.activation(v_t[0][:, H:CCH], srr[0][:, H:CCH], Exp, scale=-1.0)
            nc.scalar.activation(slb_sb[:], slps[:, 0:NBLK], Ident,
                                 bias=bvec_sb[:, 1:2])
            nc.scalar.activation(v_t[1][:, 0:H], srr[1][:, 0:H], Exp, scale=-1.0)
            nc.scalar.activation(v_t[1][:, H:CCH], srr[1][:, H:CCH], Exp, scale=-1.0)
        elif K_UFIRST:
            nc.scalar.activation(u_sb[:], slps[:, 0:NBLK], Exp, scale=-1.0,
                                 bias=bvec_sb[:, 0:1])
            nc.scalar.activation(slb_sb[:], slps[:, 0:NBLK], Ident,
                                 bias=bvec_sb[:, 1:2])
            nc.scalar.activation(v_t[0], srr[0][:], Exp, scale=-1.0)
            nc.scalar.activation(v_t[1], srr[1][:], Exp, scale=-1.0)
        else:
            nc.scalar.activation(v_t[0], srr[0][:], Exp, scale=-1.0)
            nc.scalar.activation(u_sb[:], slps[:, 0:NBLK], Exp, scale=-1.0,
                                 bias=bvec_sb[:, 0:1])
            nc.scalar.activation(slb_sb[:], slps[:, 0:NBLK], Ident,
                                 bias=bvec_sb[:, 1:2])
            nc.scalar.activation(v_t[1], srr[1][:], Exp, scale=-1.0)
        if K_S16:
            s16_0 = const_pool.tile([P, CCH], f16)
            nc.scalar.activation(s16_0[:], srr[0][:],
                                 mybir.ActivationFunctionType.Copy)

        out4 = out.rearrange("(A r d) c -> A r d c", r=P, d=1)

        for rbp in range(NBLK // 2):
            att2 = att_pool.tile([P, 2 * N], f16, tag="att")
            for half in range(2):
                rb = 2 * rbp + half
                for cc in range(NCCH):
                    a_t = adj_tiles[(rb, cc)]
                    if cc == 0 and rb in s16_rbs:
                        seg = att2[:, half * N : half * N + CCH]
                        nc.scalar.activation(seg, s16_0[:], Sig,
                                             bias=slb_sb[:, rb : rb + 1])
                        nc.vector.tensor_mul(seg, seg, a_t[:])
                        continue
                    if cc < 2:
                        if K_WIDE and (cc == 1 and rb not in s16_rbs):
                            continue
                        if cc == 1 and rb in s16_rbs:
                            seg = att2[:, half * N + CCH : half * N + 2 * CCH]
                            nc.vector._custom_dve(
                                fused_op, out=seg, in0=v_t[1], in1=a_t[:],
                                s0=u_sb[:, rb : rb + 1], s1=RC0, imm2=RC1,
                            )
                            continue
                        width = 2 * CCH if K_WIDE else CCH
                        seg = att2[:, half * N + cc * CCH : half * N + cc * CCH + width]
                        vin = vw[:, cc * CCH : cc * CCH + width]
                        if rb < 2 and K_RAMP_SPLIT:
                            # ramp: halves so the first ops start as soon as
                            # the matching v-exp piece lands
                            hw = width // 2
                            for q in range(2):
                                hs = slice(q * hw, (q + 1) * hw)
                                nc.vector._custom_dve(
                                    fused_op, out=seg[:, hs], in0=vin[:, hs],
                                    in1=a_t[:, hs],
                                    s0=u_sb[:, rb : rb + 1], s1=RC0, imm2=RC1,
                                )
                        else:
                            nc.vector._custom_dve(
                                fused_op, out=seg, in0=vin, in1=a_t[:],
                                s0=u_sb[:, rb : rb + 1], s1=RC0, imm2=RC1,
                            )
                    else:
                        seg = att2[:, half * N + cc * CCH : half * N + (cc + 1) * CCH]
                        nc.scalar.activation(seg, srr[cc][:], Sig,
                                             bias=slb_sb[:, rb : rb + 1])
                        if (rb, cc) in POOL_MULT:
                            nc.gpsimd.tensor_mul(seg, seg, a_t[:])
                        else:
                            nc.vector.tensor_mul(seg, seg, a_t[:])
            in4 = att2[:].rearrange("p (d b n) -> p d b n", d=1, b=2)
            if K_PREP:
                dma_sem = nc.alloc_semaphore(f"kvw{rbp}")
                nc.gpsimd.kv_writeback(
                    out4[2 * rbp : 2 * rbp + 2, :, :, :], in4, zidx[:],
                    prepare_only=True, sem=dma_sem)
                nc.gpsimd.trigger_dma(count=1)
            else:
                nc.gpsimd.kv_writeback(
                    out4[2 * rbp : 2 * rbp + 2, :, :, :], in4, zidx[:])

    nc.compile()
    return nc


def kernel(x, adj, W, b):
    global _nc
    x = np.ascontiguousarray(np.asarray(x, dtype=np.float32))
    adj = np.asarray(adj, dtype=np.float32)
    W = np.asarray(W, dtype=np.float32).reshape(2 * F)
    b = np.float32(np.asarray(b).reshape(()))

    if _nc is None:
        _nc = _build()

    xt_np = np.ascontiguousarray(x.T.astype(np.float16))
    wl_np = np.ascontiguousarray(W[:F, None].astype(np.float16))
    wrep_np = np.ascontiguousarray(
        np.broadcast_to(W[F:, None].astype(np.float16), (F, P))
    )
    bvec_np = np.stack([np.full(P, -b), np.full(P, b)], axis=1).astype(np.float32)

    in_maps = []
    for k in range(NCORES):
        rows = slice(k * RPC, (k + 1) * RPC)
        adj_rows = adj[rows]
        im = {
            "adj8": np.ascontiguousarray(np.rint(adj_rows * 255.0).astype(np.uint8)),
            "adj16": np.ascontiguousarray((adj_rows * 255.0).astype(np.float16)),
            "xt": xt_np,
            "xlt": np.ascontiguousarray(x[rows].T.astype(np.float16)),
            "wl": wl_np,
            "wrep": wrep_np,
            "bvec": bvec_np,
        }
        in_maps.append(im)

    import time

    from concourse.bass_utils import run_bass_kernel_spmd

    res = None
    for attempt in range(4):
        try:
            res = run_bass_kernel_spmd(_nc, in_maps, core_ids=list(range(NCORES)))
            break
        except Exception:
            # transient device wedges clear after a short wait; retry
            if attempt == 3:
                raise
            time.sleep(40 * (attempt + 1))
    scale = np.float32(1.0 / 255.0)
    return np.concatenate(
        [np.asarray(r["out"], dtype=np.float32) * scale for r in res.results], axis=0
    )


# revision 36
# speedup vs baseline: 1.0070x; 1.0070x over previous
"""DenseAtt GNN message-passing kernel for Trainium2 (8 NeuronCores).

Computes out = adj * sigmoid(s_left[:, None] + s_right[None, :] + b)
with s_left = x @ W[:F], s_right = x @ W[F:], for x [N, F], adj [N, N].

Sharding: 1D row partition of adj / out across the 8 cores (1024 rows each).

Per-core design (32 tiles of [128, 2048], graded by the Tile cost model):

  sigmoid(sl + sr) = 1 / (1 + u*v) with u = e^-(sl+b) (per-row scalar) and
  v = e^-sr (per-column vector), so a tile can be produced EITHER by the
  ACT engine's sigmoid table (input: s_right replicated in PSUM, per-row
  bias) or by a single custom DVE op

      out = adj8 * recip_1nr(v*u + 1)

  (BITWISE_NOT fp32 exponent-flip seed + one Newton step, ~1.7e-3 rel err)
  which folds denominator, reciprocal and the adj multiply into one 1x DVE
  pass. PSUM only holds two [128, 2048] f32 s_right-replicated tiles, so:

  - chunks 0-1 (cols 0..4095): fused DVE path; their srr tiles are consumed
    by ACT Exp into SBUF f16 v-tiles during the early Exp-table phase, then
    recycled. One DVE op per tile, no ACT, no Pool.
  - chunks 2-3 (cols 4096..8191): srr stays resident in PSUM; ACT sigmoid
    (bias = sl+b) writes f16 att in place, then the adj multiply runs on
    DVE at 2x (f16-staged adj, 8 tiles) or on Pool (u8 adj, 8 tiles) to
    balance the DVE and Pool chains.

  adj is staged u8 (round(255*adj), quant err ~2e-3 abs) except the 8
  DVE-mult tiles which are staged f16 (255*adj) so every operand of the 2x
  TensorTensor is 2-byte. All paths produce 255*adj*att in f16; one
  kv_writeback per row-block pair (batch=2, ncn=8192, stripe-descriptor
  pricing) returns them to HBM and the host upcasts + rescales by 1/255.
  Measured (TimelineSim, the grading model): 60632 ns vs 74876 baseline.

  s_left comes from 8 tiny PE matmuls against the core's own x^T slice;
  ACT Exp/Identity (every table set) turn it into u and the sigmoid bias,
  so the program needs exactly one Exp->Sigmoid table switch.
"""

import sys

import numpy as np

sys.path.insert(0, "/opt/trn_rl_repo")

N = 8192
F = 128
NCORES = 8
RPC = N // NCORES  # rows per core: 1024
P = 128
NBLK = RPC // P  # row blocks per core: 8
CCH = 2048
NCCH = N // CCH  # 4 column chunks

import os

# chunks 0-1: fused DVE path. chunks 2-3: sigmoid; mult engine per (rb, cc):
_POOL_SETS = {
    6: {(0, 2), (2, 2), (4, 2), (1, 3), (3, 3), (5, 3)},
    7: {(0, 2), (2, 2), (4, 2), (6, 2), (1, 3), (3, 3), (5, 3)},
    8: {(0, 2), (2, 2), (4, 2), (6, 2), (1, 3), (3, 3), (5, 3), (7, 3)},
    9: {(0, 2), (2, 2), (4, 2), (6, 2), (7, 2), (1, 3), (3, 3), (5, 3), (7, 3)},
    10: {(0, 2), (2, 2), (4, 2), (6, 2), (7, 2), (1, 3), (3, 3), (5, 3), (7, 3), (5, 2)},
}
POOL_MULT = _POOL_SETS[int(os.environ.get("K_POOLN", "8"))]
K_RAMP_SPLIT = int(os.environ.get("K_RAMP_SPLIT", "0"))
K_WIDE = int(os.environ.get("K_WIDE", "1"))  # one [128,4096] fused op per rb
K_ATT_BUFS = int(os.environ.get("K_ATT_BUFS", "3"))
# last K_S16 row-blocks' chunk-0 tiles switch from the fused path to sigmoid
# read from an SBUF f16 copy of srr0 (+ DVE f16 mult) to offload DVE onto
# ACT's tail slack
K_S16 = int(os.environ.get("K_S16", "0"))
# u-first ramp: xlt loads first, s_left matmuls precede srr0 on PE, u-exp
# precedes v0 on ACT -- shortens the chain gating every fused op
K_UFIRST = int(os.environ.get("K_UFIRST", "0"))
# swapped roles: sigmoid chunks 0-1 (srr pinned), fused chunks 2-3; fused
# tiles emitted first per row-block so DVE's queue head is never a mult
# waiting on a late sigmoid
K_SWAP = int(os.environ.get("K_SWAP", "0"))
# prepare_only writebacks: SWDGE descriptor-gen runs during Pool's idle
# ramp (deps demoted to the trigger), trigger fires the DMA when data lands
K_PREP = int(os.environ.get("K_PREP", "0"))
POOL_MULT_SWAP = {(0, 0), (2, 0), (4, 0), (6, 0), (1, 1), (3, 1), (5, 1), (7, 1)}

# 1-Newton reciprocal constants (equioscillating over the x*bitcast(~x)
# seed interval [-4.5, -4]; max rel err 1.73e-3)
RC0, RC1 = -0.23549792, 2.0017324

_nc = None
_FUSED = None


def _register_fused_op():
    """Register the custom DVE op  out = Src1 * recip_1nr(Src0*C0 + 1).

    C0 carries the per-partition u scalar; C1/C2 the reciprocal constants.
    The BITWISE_NOT seed operates on the internal fp32 value of z, so in0
    may be f16 and in1 u8.
    """
    global _FUSED
    if _FUSED is not None:
        return _FUSED
    import concourse.dve_ops as dve_ops
    from concourse.dve_spec import AluOp, Bin, C0, C1, C2, One, Spec, Src0, Src1, lower
    from concourse.dve_uop import DveOpSpec

    _z = Src0 * C0 + One
    _nz = Bin(AluOp.BITWISE_NOT, _z, _z)
    _w0 = _nz * C1
    _w1 = _w0 * (C2 - _z * _w0)

    def _ref(in0, in1, c0, c1, c2):
        z = (in0.astype(np.float32) * c0 + 1.0).astype(np.float32)
        nz = (~z.view(np.int32)).view(np.float32)
        w0 = nz * c1
        w1 = w0 * (c2 - z * w0)
        return in1.astype(np.float32) * w1

    spec = Spec(body=Src1 * _w1, reference=_ref)
    name = "FUSED_SIG_MUL"
    row = 17
    shas = {}
    for ver in ("v3", "v4"):
        uops = lower(spec, ver=ver)
        shas[ver] = DveOpSpec(name=name, opcode=row, uops=uops, rd1_en=True).sha(ver)
    op = dve_ops.DveOp(name, spec, subdim=False, uops_sha=shas)
    if not any(o.name == name for o in dve_ops.OPS):
        dve_ops.OPS.append(op)
    dve_ops.CUSTOM_DVE_SPECS[name] = spec
    dve_ops._SUB_OPCODE_FOR_NAME[name] = row
    _FUSED = op
    return op


def _build_swap():
    """Swapped-role build: sigmoid on chunks 0-1 (srr0/1 pinned in PSUM),
    fused DVE path on chunks 2-3 (v2/v3 f16 in SBUF). Exp phase = v2, u,
    slb, v3; PSUM rotation slps->srr2->srr3->srr0(pinned)->srr1(pinned).
    Per row-block the wide fused op is emitted before the sigmoid tiles."""
    from contextlib import ExitStack

    import concourse.tile as tile
    from concourse import bacc, mybir

    fused_op = _register_fused_op()

    f32 = mybir.dt.float32
    f16 = mybir.dt.float16
    u8 = mybir.dt.uint8

    nc = bacc.Bacc("TRN2", target_bir_lowering=False, debug=False,
                   enable_asserts=True, num_devices=NCORES)

    adj8 = nc.dram_tensor("adj8", [RPC, N], u8, kind="ExternalInput").ap()
    adj16 = nc.dram_tensor("adj16", [RPC, N], f16, kind="ExternalInput").ap()
    xt = nc.dram_tensor("xt", [F, N], f16, kind="ExternalInput").ap()
    xlt = nc.dram_tensor("xlt", [F, RPC], f16, kind="ExternalInput").ap()
    wl = nc.dram_tensor("wl", [F, 1], f16, kind="ExternalInput").ap()
    wrep = nc.dram_tensor("wrep", [F, P], f16, kind="ExternalInput").ap()
    bvec = nc.dram_tensor("bvec", [P, 2], f32, kind="ExternalInput").ap()
    out = nc.dram_tensor("out", [RPC, N], f16, kind="ExternalOutput").ap()

    Sig = mybir.ActivationFunctionType.Sigmoid
    Exp = mybir.ActivationFunctionType.Exp
    Ident = mybir.ActivationFunctionType.Identity

    with tile.TileContext(nc) as tc, ExitStack() as ctx:
        const_pool = ctx.enter_context(tc.tile_pool(name="const", bufs=1))
        xt_pool = ctx.enter_context(tc.tile_pool(name="xt", bufs=2))
        v_pool = ctx.enter_context(tc.tile_pool(name="v", bufs=1))
        adj8_pool = ctx.enter_context(tc.tile_pool(name="adj8", bufs=8))
        adj16_pool = ctx.enter_context(tc.tile_pool(name="adj16", bufs=6))
        att_pool = ctx.enter_context(tc.tile_pool(name="att", bufs=K_ATT_BUFS))
        ps_pool = ctx.enter_context(tc.tile_pool(name="ps", bufs=2, space="PSUM"))

        adj_tiles = {}

        def load_adj(rb, cc):
            if cc == 2:
                t = adj8_pool.tile([P, 2 * CCH], u8, tag="a8", name="a8")
                nc.sync.dma_start(t[:], adj8[rb * P : (rb + 1) * P, 2 * CCH : 4 * CCH])
                adj_tiles[(rb, 2)] = t
                adj_tiles[(rb, 3)] = t
                return
            if cc == 3:
                return
            cols = slice(cc * CCH, (cc + 1) * CCH)
            if (rb, cc) not in POOL_MULT_SWAP:
                t = adj16_pool.tile([P, CCH], f16, tag="a16", name="a16")
                nc.sync.dma_start(t[:], adj16[rb * P : (rb + 1) * P, cols])
            else:
                t = adj8_pool.tile([P, CCH], u8, tag="a8", name="a8")
                nc.sync.dma_start(t[:], adj8[rb * P : (rb + 1) * P, cols])
            adj_tiles[(rb, cc)] = t

        # DMA order: xt2+wrep gate srr2 -> v2 (the first fused ops),
        # xlt/wl gate u; rb0/rb1 adj tiles slot between the xt chunks.
        xt_t = [None] * NCCH
        xt_t[2] = xt_pool.tile([F, CCH], f16, tag="xt", name="xt_sb")
        nc.sync.dma_start(xt_t[2][:], xt[:, 2 * CCH : 3 * CCH])
        wrep_sb = const_pool.tile([F, P], f16)
        nc.sync.dma_start(wrep_sb[:], wrep)
        xlt_sb = const_pool.tile([F, RPC], f16)
        nc.sync.dma_start(xlt_sb[:], xlt)
        wl_sb = const_pool.tile([F, 1], f16)
        nc.sync.dma_start(wl_sb[:], wl)
        load_adj(0, 2)
        xt_t[3] = xt_pool.tile([F, CCH], f16, tag="xt", name="xt_sb")
        nc.sync.dma_start(xt_t[3][:], xt[:, 3 * CCH : 4 * CCH])
        load_adj(1, 2)
        xt_t[0] = xt_pool.tile([F, CCH], f16, tag="xt", name="xt_sb")
        nc.sync.dma_start(xt_t[0][:], xt[:, 0:CCH])
        load_adj(0, 0)
        xt_t[1] = xt_pool.tile([F, CCH], f16, tag="xt", name="xt_sb")
        nc.sync.dma_start(xt_t[1][:], xt[:, CCH : 2 * CCH])
        bvec_sb = const_pool.tile([P, 2], f32)
        nc.sync.dma_start(bvec_sb[:], bvec)
        load_adj(0, 1)
        for rb in range(1, NBLK):
            for cc in (2, 0, 1):
                if (rb, cc) not in adj_tiles:
                    load_adj(rb, cc)

        zidx = const_pool.tile([P, 2], mybir.dt.int32)
        nc.vector.memset(zidx, 0.0)

        # PSUM: slps(b0), srr2(b1), srr3(b0 after u/slb), srr0(b1 after
        # v2-exp), srr1(b0 after v3-exp); srr0/srr1 stay pinned.
        slps = ps_pool.tile([P, CCH], f32, tag="ps")
        srr = [None] * NCCH
        srr[2] = ps_pool.tile([P, CCH], f32, tag="ps", name="srr")
        for i in range(CCH // 512):
            nc.tensor.matmul(
                srr[2][:, i * 512 : (i + 1) * 512], wrep_sb[:],
                xt_t[2][:, i * 512 : (i + 1) * 512])
        for c in range(NBLK):
            nc.tensor.matmul(
                slps[:, c : c + 1], xlt_sb[:, c * P : (c + 1) * P], wl_sb[:])
        for cc in (3, 0, 1):
            srr[cc] = ps_pool.tile([P, CCH], f32, tag="ps", name="srr")
            for i in range(CCH // 512):
                nc.tensor.matmul(
                    srr[cc][:, i * 512 : (i + 1) * 512], wrep_sb[:],
                    xt_t[cc][:, i * 512 : (i + 1) * 512])

        vw = v_pool.tile([P, 2 * CCH], f16, tag="v", name="v_sb")
        u_sb = const_pool.tile([P, NBLK], f32)
        slb_sb = const_pool.tile([P, NBLK], f32)
        nc.scalar.activation(vw[:, 0:CCH], srr[2][:], Exp, scale=-1.0)
        nc.scalar.activation(u_sb[:], slps[:, 0:NBLK], Exp, scale=-1.0,
                             bias=bvec_sb[:, 0:1])
        nc.scalar.activation(slb_sb[:], slps[:, 0:NBLK], Ident,
                             bias=bvec_sb[:, 1:2])
        nc.scalar.activation(vw[:, CCH : 2 * CCH], srr[3][:], Exp, scale=-1.0)

        out4 = out.rearrange("(A r d) c -> A r d c", r=P, d=1)

        for rbp in range(NBLK // 2):
            att2 = att_pool.tile([P, 2 * N], f16, tag="att")
            for half in range(2):
                rb = 2 * rbp + half
                # fused first: ramp rbs split per chunk (cc2 gated only by
                # the early v2-exp); steady-state rbs use one wide op
                if rb < 2:
                    for q in range(2):
                        seg = att2[:, half * N + (2 + q) * CCH : half * N + (3 + q) * CCH]
                        nc.vector._custom_dve(
                            fused_op, out=seg,
                            in0=vw[:, q * CCH : (q + 1) * CCH],
                            in1=adj_tiles[(rb, 2)][:, q * CCH : (q + 1) * CCH],
                            s0=u_sb[:, rb : rb + 1], s1=RC0, imm2=RC1)
                else:
                    seg = att2[:, half * N + 2 * CCH : half * N + 4 * CCH]
                    nc.vector._custom_dve(
                        fused_op, out=seg, in0=vw[:], in1=adj_tiles[(rb, 2)][:],
                        s0=u_sb[:, rb : rb + 1], s1=RC0, imm2=RC1)
                for cc in (0, 1):
                    segc = att2[:, half * N + cc * CCH : half * N + (cc + 1) * CCH]
                    nc.scalar.activation(segc, srr[cc][:], Sig,
                                         bias=slb_sb[:, rb : rb + 1])
                    if (rb, cc) in POOL_MULT_SWAP:
                        nc.gpsimd.tensor_mul(segc, segc, adj_tiles[(rb, cc)][:])
                    else:
                        nc.vector.tensor_mul(segc, segc, adj_tiles[(rb, cc)][:])
            in4 = att2[:].rearrange("p (d b n) -> p d b n", d=1, b=2)
            nc.gpsimd.kv_writeback(out4[2 * rbp : 2 * rbp + 2, :, :, :], in4, zidx[:])

    nc.compile()
    return nc


def _build():
    from contextlib import ExitStack

    import concourse.tile as tile
    from concourse import bacc, mybir

    if K_SWAP:
        return _build_swap()

    fused_op = _register_fused_op()

    f32 = mybir.dt.float32
    f16 = mybir.dt.float16
    u8 = mybir.dt.uint8

    nc = bacc.Bacc(
        "TRN2",
        target_bir_lowering=False,
        debug=False,
        enable_asserts=True,
        num_devices=NCORES,
    )

    adj8 = nc.dram_tensor("adj8", [RPC, N], u8, kind="ExternalInput").ap()
    adj16 = nc.dram_tensor("adj16", [RPC, N], f16, kind="ExternalInput").ap()
    xt = nc.dram_tensor("xt", [F, N], f16, kind="ExternalInput").ap()
    xlt = nc.dram_tensor("xlt", [F, RPC], f16, kind="ExternalInput").ap()
    wl = nc.dram_tensor("wl", [F, 1], f16, kind="ExternalInput").ap()
    wrep = nc.dram_tensor("wrep", [F, P], f16, kind="ExternalInput").ap()
    bvec = nc.dram_tensor("bvec", [P, 2], f32, kind="ExternalInput").ap()
    out = nc.dram_tensor("out", [RPC, N], f16, kind="ExternalOutput").ap()

    Sig = mybir.ActivationFunctionType.Sigmoid
    Exp = mybir.ActivationFunctionType.Exp
    Ident = mybir.ActivationFunctionType.Identity

    with tile.TileContext(nc) as tc, ExitStack() as ctx:
        const_pool = ctx.enter_context(tc.tile_pool(name="const", bufs=1))
        xt_pool = ctx.enter_context(tc.tile_pool(name="xt", bufs=2))
        v_pool = ctx.enter_context(tc.tile_pool(name="v", bufs=2))
        adj8_pool = ctx.enter_context(tc.tile_pool(name="adj8", bufs=8))
        adj16_pool = ctx.enter_context(tc.tile_pool(name="adj16", bufs=6))
        att_pool = ctx.enter_context(tc.tile_pool(name="att", bufs=K_ATT_BUFS))
        ps_pool = ctx.enter_context(tc.tile_pool(name="ps", bufs=2, space="PSUM"))

        # adj tile loader: tiles are issued in an explicit early-prefetch
        # order (interleaved with the xt loads below) so the first fused op
        # is gated by v0, not by its adj DMA sitting behind 9us of loads.
        adj_tiles = {}

        s16_rbs = set(range(NBLK - K_S16, NBLK))

        def load_adj(rb, cc):
            if K_WIDE and cc == 0 and rb not in s16_rbs:
                # one [P, 2*CCH] u8 tile covering both fused chunks
                t = adj8_pool.tile([P, 2 * CCH], u8, tag="a8", name="a8")
                nc.sync.dma_start(t[:], adj8[rb * P : (rb + 1) * P, 0 : 2 * CCH])
                adj_tiles[(rb, 0)] = t
                adj_tiles[(rb, 1)] = t
                return
            if K_WIDE and cc == 1 and rb not in s16_rbs:
                return
            cols = slice(cc * CCH, (cc + 1) * CCH)
            sig_dve = (cc >= 2 and (rb, cc) not in POOL_MULT) or (
                cc == 0 and rb in s16_rbs
            )
            if sig_dve:
                t = adj16_pool.tile([P, CCH], f16, tag="a16", name="a16")
                nc.sync.dma_start(t[:], adj16[rb * P : (rb + 1) * P, cols])
            else:
                t = adj8_pool.tile([P, CCH], u8, tag="a8", name="a8")
                nc.sync.dma_start(t[:], adj8[rb * P : (rb + 1) * P, cols])
            adj_tiles[(rb, cc)] = t

        # DMA order = DMA-engine service order: xt0+wrep gate srr0 -> v0 (the
        # first fused tiles), xlt/wl gate u and the sigmoid biases; rb0's adj
        # tiles slot between the remaining xt chunks.
        xt_t = [None] * NCCH
        if K_UFIRST:
            xlt_sb = const_pool.tile([F, RPC], f16)
            nc.sync.dma_start(xlt_sb[:], xlt)
            wl_sb = const_pool.tile([F, 1], f16)
            nc.sync.dma_start(wl_sb[:], wl)
            xt_t[0] = xt_pool.tile([F, CCH], f16, tag="xt", name="xt_sb")
            nc.sync.dma_start(xt_t[0][:], xt[:, 0:CCH])
            wrep_sb = const_pool.tile([F, P], f16)
            nc.sync.dma_start(wrep_sb[:], wrep)
        else:
            xt_t[0] = xt_pool.tile([F, CCH], f16, tag="xt", name="xt_sb")
            nc.sync.dma_start(xt_t[0][:], xt[:, 0:CCH])
            wrep_sb = const_pool.tile([F, P], f16)
            nc.sync.dma_start(wrep_sb[:], wrep)
            xlt_sb = const_pool.tile([F, RPC], f16)
            nc.sync.dma_start(xlt_sb[:], xlt)
            wl_sb = const_pool.tile([F, 1], f16)
            nc.sync.dma_start(wl_sb[:], wl)
        # bvec is 8 bytes and gates u (and therefore every fused op) -- it
        # must land before the bulk adj/xt traffic
        bvec_sb = const_pool.tile([P, 2], f32)
        nc.sync.dma_start(bvec_sb[:], bvec)
        load_adj(0, 0)
        xt_t[1] = xt_pool.tile([F, CCH], f16, tag="xt", name="xt_sb")
        nc.sync.dma_start(xt_t[1][:], xt[:, CCH : 2 * CCH])
        load_adj(0, 1)
        xt_t[2] = xt_pool.tile([F, CCH], f16, tag="xt", name="xt_sb")
        nc.sync.dma_start(xt_t[2][:], xt[:, 2 * CCH : 3 * CCH])
        load_adj(1, 0)
        xt_t[3] = xt_pool.tile([F, CCH], f16, tag="xt", name="xt_sb")
        nc.sync.dma_start(xt_t[3][:], xt[:, 3 * CCH : 4 * CCH])
        load_adj(0, 2)
        load_adj(1, 1)
        load_adj(0, 3)
        for rb in range(1, NBLK):
            for cc in range(NCCH):
                if (rb, cc) not in adj_tiles:
                    load_adj(rb, cc)

        zidx = const_pool.tile([P, 2], mybir.dt.int32)
        nc.vector.memset(zidx, 0.0)

        # PSUM buffer rotation: srr0(b0), slps(b1), srr1(b0? no - srr0 must
        # persist until v0-exp) -- allocation order srr0, slps, srr1, srr2,
        # srr3 with bufs=2: srr1 reuses b0 after v0-exp frees srr0, srr2
        # reuses b1 after u/slb free slps, srr3 reuses b0 after v1-exp.
        # srr2/srr3 persist for the sigmoid tiles.
        # PE order: srr0 first (gates v0 -> the first fused ops), then
        # s_left, then srr1..3.
        srr = [None] * NCCH
        if K_UFIRST:
            slps = ps_pool.tile([P, CCH], f32, tag="ps")
            for c in range(NBLK):
                nc.tensor.matmul(
                    slps[:, c : c + 1], xlt_sb[:, c * P : (c + 1) * P], wl_sb[:]
                )
            srr[0] = ps_pool.tile([P, CCH], f32, tag="ps", name="srr")
            for i in range(CCH // 512):
                nc.tensor.matmul(
                    srr[0][:, i * 512 : (i + 1) * 512],
                    wrep_sb[:],
                    xt_t[0][:, i * 512 : (i + 1) * 512],
                )
        else:
            srr[0] = ps_pool.tile([P, CCH], f32, tag="ps", name="srr")
            for i in range(CCH // 512):
                nc.tensor.matmul(
                    srr[0][:, i * 512 : (i + 1) * 512],
                    wrep_sb[:],
                    xt_t[0][:, i * 512 : (i + 1) * 512],
                )
            slps = ps_pool.tile([P, CCH], f32, tag="ps")
            for c in range(NBLK):
                nc.tensor.matmul(
                    slps[:, c : c + 1], xlt_sb[:, c * P : (c + 1) * P], wl_sb[:]
                )
        for cc in range(1, NCCH):
            srr[cc] = ps_pool.tile([P, CCH], f32, tag="ps", name="srr")
            for i in range(CCH // 512):
                nc.tensor.matmul(
                    srr[cc][:, i * 512 : (i + 1) * 512],
                    wrep_sb[:],
                    xt_t[cc][:, i * 512 : (i + 1) * 512],
                )

        # Exp-table phase, ACT order: v0 first (gates the first fused ops) in
        # 1024-halves so the ramp's fused sub-ops unblock ASAP, then u (gates
        # every fused op), slb, then v1.
        H = CCH // 2
        vw = v_pool.tile([P, 2 * CCH], f16, tag="v", name="v_sb")
        v_t = [vw[:, 0:CCH], vw[:, CCH : 2 * CCH]]
        u_sb = const_pool.tile([P, NBLK], f32)
        slb_sb = const_pool.tile([P, NBLK], f32)
        s16_0 = None
        if K_RAMP_SPLIT:
            nc.scalar.activation(v_t[0][:, 0:H], srr[0][:, 0:H], Exp, scale=-1.0)
            nc.scalar.activation(u_sb[:], slps[:, 0:NBLK], Exp, scale=-1.0,
                                 bias=bvec_sb[:, 0:1])
            nc.scalar.activation(v_t[0][:, H:CCH], srr[0][:, H:CCH], Exp, scale=-1.0)
            nc.scalar.activation(slb_sb[:], slps[:, 0:NBLK], Ident,
                                 bias=bvec_sb[:, 1:2])
            nc.scalar.activation(v_t[1][:, 0:H], srr[1][:, 0:H], Exp, scale=-1.0)
            nc.scalar.activation(v_t[1][:, H:CCH], srr[1][:, H:CCH], Exp, scale=-1.0)
        elif K_UFIRST:
            nc.scalar.activation(u_sb[:], slps[:, 0:NBLK], Exp, scale=-1.0,
                                 bias=bvec_sb[:, 0:1])
            nc.scalar.activation(slb_sb[:], slps[:, 0:NBLK], Ident,
                                 bias=bvec_sb[:, 1:2])
            nc.scalar.activation(v_t[0], srr[0][:], Exp, scale=-1.0)
            nc.scalar.activation(v_t[1], srr[1][:], Exp, scale=-1.0)
        else:
            nc.scalar.activation(v_t[0], srr[0][:], Exp, scale=-1.0)
            nc.scalar.activation(u_sb[:], slps[:, 0:NBLK], Exp, scale=-1.0,
                                 bias=bvec_sb[:, 0:1])
            nc.scalar.activation(slb_sb[:], slps[:, 0:NBLK], Ident,
                                 bias=bvec_sb[:, 1:2])
            nc.scalar.activation(v_t[1], srr[1][:], Exp, scale=-1.0)
        if K_S16:
            s16_0 = const_pool.tile([P, CCH], f16)
            nc.scalar.activation(s16_0[:], srr[0][:],
                                 mybir.ActivationFunctionType.Copy)

        out4 = out.rearrange("(A r d) c -> A r d c", r=P, d=1)

        for rbp in range(NBLK // 2):
            att2 = att_pool.tile([P, 2 * N], f16, tag="att")
            for half in range(2):
                rb = 2 * rbp + half
                for cc in range(NCCH):
                    a_t = adj_tiles[(rb, cc)]
                    if cc == 0 and rb in s16_rbs:
                        seg = att2[:, half * N : half * N + CCH]
                        nc.scalar.activation(seg, s16_0[:], Sig,
                                             bias=slb_sb[:, rb : rb + 1])
                        nc.vector.tensor_mul(seg, seg, a_t[:])
                        continue
                    if cc < 2:
                        if K_WIDE and (cc == 1 and rb not in s16_rbs):
                            continue
                        if cc == 1 and rb in s16_rbs:
                            seg = att2[:, half * N + CCH : half * N + 2 * CCH]
                            nc.vector._custom_dve(
                                fused_op, out=seg, in0=v_t[1], in1=a_t[:],
                                s0=u_sb[:, rb : rb + 1], s1=RC0, imm2=RC1,
                            )
                            continue
                        width = 2 * CCH if K_WIDE else CCH
                        seg = att2[:, half * N + cc * CCH : half * N + cc * CCH + width]
                        vin = vw[:, cc * CCH : cc * CCH + width]
                        if rb < 2 and K_RAMP_SPLIT:
                            # ramp: halves so the first ops start as soon as
                            # the matching v-exp piece lands
                            hw = width // 2
                            for q in range(2):
                                hs = slice(q * hw, (q + 1) * hw)
                                nc.vector._custom_dve(
                                    fused_op, out=seg[:, hs], in0=vin[:, hs],
                                    in1=a_t[:, hs],
                                    s0=u_sb[:, rb : rb + 1], s1=RC0, imm2=RC1,
                                )
                        else:
                            nc.vector._custom_dve(
                                fused_op, out=seg, in0=vin, in1=a_t[:],
                                s0=u_sb[:, rb : rb + 1], s1=RC0, imm2=RC1,
                            )
                    else:
                        seg = att2[:, half * N + cc * CCH : half * N + (cc + 1) * CCH]
                        nc.scalar.activation(seg, srr[cc][:], Sig,
                                             bias=slb_sb[:, rb : rb + 1])
                        if (rb, cc) in POOL_MULT:
                            nc.gpsimd.tensor_mul(seg, seg, a_t[:])
                        else:
                            nc.vector.tensor_mul(seg, seg, a_t[:])
            in4 = att2[:].rearrange("p (d b n) -> p d b n", d=1, b=2)
            if K_PREP:
                dma_sem = nc.alloc_semaphore(f"kvw{rbp}")
                nc.gpsimd.kv_writeback(
                    out4[2 * rbp : 2 * rbp + 2, :, :, :], in4, zidx[:],
                    prepare_only=True, sem=dma_sem)
                nc.gpsimd.trigger_dma(count=1)
            else:
                nc.gpsimd.kv_writeback(
                    out4[2 * rbp : 2 * rbp + 2, :, :, :], in4, zidx[:])

    nc.compile()
    return nc


def kernel(x, adj, W, b):
    global _nc
    x = np.ascontiguousarray(np.asarray(x, dtype=np.float32))
    adj = np.asarray(adj, dtype=np.float32)
    W = np.asarray(W, dtype=np.float32).reshape(2 * F)
    b = np.float32(np.asarray(b).reshape(()))

    if _nc is None:
        _nc = _build()

    xt_np = np.ascontiguousarray(x.T.astype(np.float16))
    wl_np = np.ascontiguousarray(W[:F, None].astype(np.float16))
    wrep_np = np.ascontiguousarray(
        np.broadcast_to(W[F:, None].astype(np.float16), (F, P))
    )
    bvec_np = np.stack([np.full(P, -b), np.full(P, b)], axis=1).astype(np.float32)

    in_maps = []
    for k in range(NCORES):
        rows = slice(k * RPC, (k + 1) * RPC)
        adj_rows = adj[rows]
        im = {
            "adj8": np.ascontiguousarray(np.rint(adj_rows * 255.0).astype(np.uint8)),
            "adj16": np.ascontiguousarray((adj_rows * 255.0).astype(np.float16)),
            "xt": xt_np,
            "xlt": np.ascontiguousarray(x[rows].T.astype(np.float16)),
            "wl": wl_np,
            "wrep": wrep_np,
            "bvec": bvec_np,
        }
        in_maps.append(im)

    import time

    from concourse.bass_utils import run_bass_kernel_spmd

    res = None
    for attempt in range(4):
        try:
            res = run_bass_kernel_spmd(_nc, in_maps, core_ids=list(range(NCORES)))
            break
        except Exception:
            # transient device wedges clear after a short wait; retry
            if attempt == 3:
                raise
            time.sleep(40 * (attempt + 1))
    scale = np.float32(1.0 / 255.0)
    return np.concatenate(
        [np.asarray(r["out"], dtype=np.float32) * scale for r in res.results], axis=0
    )
